# revision 16
# baseline (speedup 1.0000x reference)
"""Demodulated 3x3 convolution Trainium2 kernel — data-stationary odd-pair
scheme.

Data-parallel over batch: 16 samples -> 8 cores x 2 samples.

Input path: HBM NHWC rows are fetched in 8-row chunks with partition =
w-PAIR (512B contiguous descriptors), Pool/SWDGE DMA casting fp32->bf16
in flight.  PE-transpose turns each row into a channel-major slab
[(q,c) 128 parts, 128 wpair cols]; DVE/ACT copies place it into a
non-circular 258-slot ring with ODD-PAIR lanes: slot sl holds image row
sl-1 as ring[(d,c), 129*sl + u] = x[row, 2u+d-1, c] (parts 0:64 = d=1
lane at col offset 0, parts 64:128 = d=0 lane at offset +1; w=-1/256
halo cols and slots 0/257 are statically zero).

Conv (data-stationary): for output row m, wpair j, out[j, (qo,f)] is
accumulated by 6 matmuls (3 ky-groups g x 2 shifts v) with lhsT =
ring[:, 129*(m+g)+v : +128] (the DATA is stationary; Ldweights are free
in the cost model) and rhs = per-sample weight tile wq[s,g,v]
[(d,c), (qo,f)] holding styled+demodulated tap W[g, kx], kx = 2v+d-qo
when 0<=kx<=2 (3 of 4 blocks per tile; 18 blocks over 6 tiles = the
provable minimum for this tap/domino structure).  N=128 per matmul,
PE/iter = 12 matmuls + 2 transposes = 1792 cyc.

Demodulation is folded into the weights: wq = W * const_k * style[c] *
demod[f] built per-sample on DVE (demod[f] broadcast across partitions
via a PE transpose of a column-replicated rsqrt).  The epilogue is a
single ACT copy Q'[j, 2, (qo,f)] PSUM fp32 -> SBUF, and the SP output
DMA writes 512B descriptors.  Conv bias is zero for this problem; a
nonzero bias selects a variant adding a K=1 matmul per row.

Per-iteration budget (cost model): PE 747ns, ACT ~690, DVE ~583,
DMA engines ~728, HWDGE 625, Pool ~335, SP ~590.
"""

import math
import os
import sys

os.environ["BASS_NEVER_TRACE"] = "1"

for _p in ("/opt/trn_rl_repo",):
    if _p not in sys.path:
        sys.path.insert(0, _p)

import numpy as np

import concourse.bass as bass
import concourse.bacc as bacc
import concourse.mybir as mybir
import concourse.tile as tile
from concourse.bass_utils import run_bass_kernel_spmd

B, H, W, CIN = 16, 256, 256, 64
LATENT, F, KK = 512, 64, 3
NCORES = 8
BS = B // NCORES  # samples per core
SLOT = 129  # ring cols per slot (u = 0..128)
NSLOT = H + 2  # slots 0..257; slot sl holds image row sl-1
RW = SLOT * NSLOT

F32 = mybir.dt.float32
BF16 = mybir.dt.bfloat16
AF = mybir.ActivationFunctionType
ALU = mybir.AluOpType

_CACHE = {}


def _build_nc(bias_nonzero):
    nc = bacc.Bacc("TRN2", target_bir_lowering=False, debug=False)

    x_h = nc.dram_tensor("x", [BS, H, W, CIN], F32, kind="ExternalInput")
    lat_h = nc.dram_tensor("lat", [BS, LATENT], F32, kind="ExternalInput")
    dw_h = nc.dram_tensor("dw", [LATENT, CIN], F32, kind="ExternalInput")
    db_h = nc.dram_tensor("db", [CIN], F32, kind="ExternalInput")
    ck_h = nc.dram_tensor("ck", [KK, KK, CIN, F], F32, kind="ExternalInput")
    bi_h = nc.dram_tensor("bi", [F], F32, kind="ExternalInput")
    id_h = nc.dram_tensor("ident", [128, 128], F32, kind="ExternalInput")
    out_h = nc.dram_tensor("out", [BS, H, W, F], F32, kind="ExternalOutput")

    const_k = math.sqrt(2.0) / math.sqrt(KK * KK * CIN)
    inv_sqrt_lat = 1.0 / math.sqrt(LATENT)

    with tile.TileContext(nc) as tc:
        with (
            tc.tile_pool(name="const", bufs=1) as cpool,
            tc.tile_pool(name="wpool", bufs=1) as wpool,
        ):
            # --- loads: ksty0+chunks on Pool; ident/lat/dw spread across
            # the three HWDGE queues so their SEQ issue overlaps ---
            identD = cpool.tile([128, 128], F32)
            nc.sync.dma_start(identD[:], id_h[:])
            latR = cpool.tile([BS, LATENT], F32)
            nc.sync.dma_start(latR[:], lat_h[:])
            dwD = cpool.tile([128, 4, CIN], F32)
            nc.scalar.dma_start(
                dwD[:], dw_h[:].rearrange("(j p) f -> p j f", p=128)
            )
            ksty0 = cpool.tile([64, 9, F], F32)

            # DVE: matmul operand copies first so style matmuls start early
            dwT = cpool.tile([128, 4, CIN], F32)
            nc.vector.tensor_copy(dwT[:], dwD[:])
            identB = cpool.tile([128, 128], BF16)
            nc.vector.tensor_copy(identB[:], identD[:])
            ones64 = cpool.tile([64, 1], F32)
            nc.vector.memset(ones64[:], 1.0)
            zero64 = cpool.tile([64, 1], F32)
            nc.vector.memset(zero64[:], 0.0)
            onesT64 = cpool.tile([64, 64], F32)
            nc.vector.memset(onesT64[:], 1.0)
            epsT = cpool.tile([64, F], F32)
            nc.vector.memset(epsT[:], 1e-8 / 64.0)

            ring = cpool.tile([128, RW], BF16)

            wq = {}  # (s, g, v) -> [128,128] bf16 lhs... rhs weight tile
            for _s in range(BS):
                for _g in range(3):
                    for _v in range(2):
                        wt = wpool.tile(
                            [128, 128], BF16, tag=f"wq{_s}_{_g}_{_v}"
                        )
                        wq[(_s, _g, _v)] = wt

            # main-loop pools open BEFORE the prologue pool so the
            # prologue's released zone is never reused
            from contextlib import ExitStack
            _mstack = ExitStack()
            stpool = _mstack.enter_context(tc.tile_pool(name="stage", bufs=4))
            opool = _mstack.enter_context(tc.tile_pool(name="onat", bufs=6))
            tpsum = _mstack.enter_context(
                tc.tile_pool(name="tpsum", bufs=2, space="PSUM"))
            qpsum = _mstack.enter_context(
                tc.tile_pool(name="qpsum", bufs=3, space="PSUM"))

            # Pool queue: ksty0 first (weight build is the prologue critical
            # path), then chunk0, then ring zeroing
            nc.gpsimd.dma_start(
                ksty0[:], ck_h[:].rearrange("ky kx c f -> c (ky kx) f")
            )
            stg0_pre = stpool.tile([128, 8, 128], BF16)
            nc.gpsimd.dma_start(
                stg0_pre[:],
                x_h[0, 0:8, :, :].rearrange("r (p q) c -> p r (q c)", p=128),
            )
            stg1_pre = stpool.tile([128, 8, 128], BF16)
            nc.gpsimd.dma_start(
                stg1_pre[:],
                x_h[0, 8:16, :, :].rearrange("r (p q) c -> p r (q c)", p=128),
            )
            # static ring zeroing: halo cols of every slot + slots 0/257
            ring_v = ring[:].rearrange("p (s c) -> p s c", c=SLOT)
            nc.gpsimd.memset(ring_v[0:64, :, 128:129], 0.0)
            nc.gpsimd.memset(ring_v[64:128, :, 0:1], 0.0)
            nc.gpsimd.memset(ring[:, 0:SLOT], 0.0)
            nc.gpsimd.memset(ring[:, SLOT * (NSLOT - 1) : SLOT * NSLOT], 0.0)

            # ---- prologue: style for both samples ----
            pro = _mstack.enter_context(tc.tile_pool(name="pro", bufs=1))
            prop = _mstack.enter_context(
                tc.tile_pool(name="prop", bufs=1, space="PSUM"))

            psLat = prop.tile([128, 4, BS], F32, tag="p")
            for jj in range(4):
                nc.tensor.transpose(
                    psLat[:, jj, :],
                    latR[:, 128 * jj : 128 * jj + 128],
                    identD[0:BS, 0:BS],
                )
            latT = pro.tile([128, 4, BS], F32)
            nc.vector.tensor_copy(latT[:], psLat[:])
            ps_style = prop.tile([CIN, BS], F32, tag="p")
            for jj in range(4):
                nc.tensor.matmul(
                    ps_style[:],
                    dwT[:, jj, :],
                    latT[:, jj, :],
                    start=(jj == 0),
                    stop=(jj == 3),
                )
            db_t = pro.tile([CIN, 1], F32)
            nc.sync.dma_start(db_t[:], db_h[:].rearrange("(c u) -> c u", u=1))
            db_s = pro.tile([CIN, 1], F32)
            nc.vector.tensor_scalar_mul(db_s[:], db_t[:], const_k)
            styleC = pro.tile([64, BS], F32)
            nc.vector.tensor_scalar(
                styleC[:],
                ps_style[:],
                inv_sqrt_lat * const_k,
                db_s[:],
                op0=ALU.mult,
                op1=ALU.add,
            )

            if bias_nonzero:
                onesRow = pro.tile([1, 128], F32)
                nc.vector.memset(onesRow[:], 1.0)
                biasPair = pro.tile([1, 128], F32)
                nc.sync.dma_start(
                    biasPair[0:1, 0:64],
                    bi_h[:].rearrange("(u c) -> u c", u=1),
                )
                nc.sync.dma_start(
                    biasPair[0:1, 64:128],
                    bi_h[:].rearrange("(u c) -> u c", u=1),
                )

            # demod chains for BOTH samples in the prologue (keeps the mid-
            # loop free of Sqrt act-table reloads): demod[f] = rsqrt(
            # sum_c K2[c,f]*style[c]^2 + eps) with K2 from ksq = ksty0^2.
            ksq = pro.tile([64, 9, F], F32)
            nc.vector.tensor_mul(ksq[:], ksty0[:], ksty0[:])
            style2 = pro.tile([64, BS], F32)
            nc.vector.tensor_mul(style2[:], styleC[:], styleC[:])

            emm = []  # per-sample style*demod outer products [c, f]
            for s in range(BS):
                ps_d = prop.tile([CIN, 1], F32, tag="p")
                for t9 in range(9):
                    nc.tensor.matmul(
                        ps_d[:], ksq[:, t9, :], style2[:, s : s + 1],
                        start=(t9 == 0), stop=False,
                    )
                nc.tensor.matmul(
                    ps_d[:], epsT[:], ones64[:], start=False, stop=True
                )
                rt = pro.tile([64, 1], F32, tag=f"rt{s}")
                nc.scalar.activation(rt[:], ps_d[:], AF.Sqrt, bias=zero64[:])
                dm = pro.tile([64, 1], F32, tag=f"dm{s}")
                nc.vector.reciprocal(dm[:], rt[:])
                # demod[f] broadcast across partitions: repl[c,f]=dm[c],
                # PE transpose -> replT[p,n]=dm[n], fused *style -> M
                repl = pro.tile([64, 64], F32, tag=f"repl{s}")
                nc.scalar.activation(repl[:], onesT64[:], AF.Copy,
                                     scale=dm[:])
                replT = prop.tile([64, 64], F32, tag="p")
                nc.tensor.transpose(replT[:], repl[:], identD[0:64, 0:64])
                mm_t = pro.tile([64, 64], F32, tag=f"M{s}")
                nc.vector.tensor_scalar_mul(
                    mm_t[:], replT[:], styleC[:, s : s + 1]
                )
                emm.append(mm_t)

            def block_ops(s, split_pool):
                """Thunks for sample s's 6 weight tiles: zero-block memsets
                + 18 fused block muls wq_block = ksty0[:,idx,:] * M[c,f]."""
                ops = []
                for g in range(3):
                    ops.append(lambda g=g: nc.vector.memset(
                        wq[(s, g, 0)][64:128, 64:128], 0.0))
                    ops.append(lambda g=g: nc.vector.memset(
                        wq[(s, g, 1)][0:64, 0:64], 0.0))
                blocks = []
                for g in range(3):
                    blocks += [
                        (wq[(s, g, 0)][0:64, 0:64], 3 * g + 1),
                        (wq[(s, g, 0)][0:64, 64:128], 3 * g + 0),
                        (wq[(s, g, 0)][64:128, 0:64], 3 * g + 0),
                        (wq[(s, g, 1)][0:64, 64:128], 3 * g + 2),
                        (wq[(s, g, 1)][64:128, 0:64], 3 * g + 2),
                        (wq[(s, g, 1)][64:128, 64:128], 3 * g + 1),
                    ]
                for i, (dst, idx) in enumerate(blocks):
                    eng = nc.gpsimd if (split_pool and i % 2) else nc.vector
                    ops.append(lambda dst=dst, idx=idx, eng=eng:
                               eng.tensor_mul(dst, ksty0[:, idx, :],
                                              emm[s][:]))
                return ops

            for op in block_ops(0, split_pool=True):
                op()

            # ---- main loop ----
            s1_pre = {}
            for s in range(BS):
                if s == 0:
                    stgs = {0: stg0_pre, 1: stg1_pre}
                    shadow = block_ops(1, split_pool=False)
                else:
                    stgs = dict(s1_pre)
                    shadow = []
                qtiles = {}
                onat_cur = None

                for t in range(131):
                    if s == 0 and t in (122, 126):
                        # prefetch sample 1's first chunks during s0's tail
                        kk = 0 if t == 122 else 1
                        stg1p = stpool.tile([128, 8, 128], BF16)
                        nc.gpsimd.dma_start(
                            stg1p[:],
                            x_h[1, 8 * kk : 8 * kk + 8, :, :].rearrange(
                                "r (p q) c -> p r (q c)", p=128
                            ),
                        )
                        s1_pre[kk] = stg1p
                    # -- input rows 2t, 2t+1 --
                    if t <= 127:
                        k, r4 = divmod(t, 4)
                        if r4 == 0 and k + 2 <= 31:
                            stgk = stpool.tile([128, 8, 128], BF16)
                            nc.gpsimd.dma_start(
                                stgk[:],
                                x_h[s, 8 * (k + 2) : 8 * (k + 3), :, :]
                                .rearrange("r (p q) c -> p r (q c)", p=128),
                            )
                            stgs[k + 2] = stgk
                            stgs.pop(k - 1, None)
                        chunk = stgs[k]
                        ptA = tpsum.tile([128, 128], BF16, tag="ptA")
                        ptB = tpsum.tile([128, 128], BF16, tag="ptB")
                        nc.tensor.transpose(
                            ptA[:], chunk[:, 2 * r4, :], identB[:]
                        )
                        nc.tensor.transpose(
                            ptB[:], chunk[:, 2 * r4 + 1, :], identB[:]
                        )
                        sc0 = SLOT * (2 * t + 1)
                        sc1 = SLOT * (2 * t + 2)
                        # DVE copies alternate PSUM zones (A,B,A) — reads
                        # of the same PSUM tile back-to-back serialize on
                        # the pipeline tail (+160ns each)
                        nc.vector.tensor_copy(
                            ring[0:64, sc0 : sc0 + 128], ptA[0:64, :]
                        )
                        nc.vector.tensor_copy(
                            ring[0:64, sc1 : sc1 + 128], ptB[0:64, :]
                        )
                        nc.vector.tensor_copy(
                            ring[64:128, sc0 + 1 : sc0 + 129], ptA[64:128, :]
                        )
                        nc.scalar.activation(
                            ring[64:128, sc1 + 1 : sc1 + 129],
                            ptB[64:128, :],
                            AF.Copy,
                        )

                    if s == 0 and shadow and 6 <= t and t % 2 == 0:
                        shadow.pop(0)()

                    # -- matmuls for row-pair jmm = t-2 --
                    jmm = t - 2
                    if 0 <= jmm <= 127:
                        Qp = qpsum.tile([128, 2, 128], F32, tag="q")
                        for r in range(2):
                            m = 2 * jmm + r
                            first = True
                            for g in range(3):
                                base = SLOT * (m + g)
                                for v in range(2):
                                    last = (
                                        g == 2 and v == 1
                                        and not bias_nonzero
                                    )
                                    nc.tensor.matmul(
                                        Qp[:, r, :],
                                        ring[:, base + v : base + v + 128],
                                        wq[(s, g, v)][:],
                                        start=first,
                                        stop=last,
                                        skip_group_check=True,
                                    )
                                    first = False
                            if bias_nonzero:
                                nc.tensor.matmul(
                                    Qp[:, r, :],
                                    onesRow[:],
                                    biasPair[:],
                                    start=False,
                                    stop=True,
                                    skip_group_check=True,
                                )
                        qtiles[jmm] = Qp

                    # -- epilogue for je = t-3 (out-DMA batched 2 iters) --
                    je = t - 3
                    if 0 <= je <= 127:
                        Qe = qtiles.pop(je)
                        if je % 2 == 0:
                            onat_cur = opool.tile([128, 4, 128], F32)
                        h = 2 * (je % 2)
                        nc.scalar.activation(
                            onat_cur[:, h : h + 2, :], Qe[:], AF.Copy
                        )
                        if je % 2 == 1:
                            nc.sync.dma_start(
                                out_h[s, 2 * je - 2 : 2 * je + 2, :, :]
                                .rearrange("r (p q) f -> p r (q f)", p=128),
                                onat_cur[:],
                            )

            _mstack.close()

    nc.compile()
    return nc


def _get_nc(bias_nonzero=False):
    key = bool(bias_nonzero)
    if key not in _CACHE:
        _CACHE[key] = _build_nc(key)
    return _CACHE[key]


def kernel(feature_map, latent, dense_w, dense_b, conv_kernel, bias):
    bias = np.ascontiguousarray(bias, dtype=np.float32)
    nc = _get_nc(bool(np.any(bias)))
    feature_map = np.ascontiguousarray(feature_map, dtype=np.float32)
    latent = np.ascontiguousarray(latent, dtype=np.float32)
    ident = np.eye(128, dtype=np.float32)
    in_maps = []
    for i in range(NCORES):
        in_maps.append(
            {
                "x": np.ascontiguousarray(feature_map[BS * i : BS * (i + 1)]),
                "lat": np.ascontiguousarray(latent[BS * i : BS * (i + 1)]),
                "dw": np.ascontiguousarray(dense_w, dtype=np.float32),
                "db": np.ascontiguousarray(dense_b, dtype=np.float32),
                "ck": np.ascontiguousarray(conv_kernel, dtype=np.float32),
                "bi": bias,
                "ident": ident,
            }
        )
    res = run_bass_kernel_spmd(nc, in_maps, core_ids=list(range(NCORES)))
    outs = [res.results[i]["out"] for i in range(NCORES)]
    full = np.concatenate(outs, axis=0)
    if getattr(res, "exec_time_ns", None):
        kernel.last_exec_time_ns = res.exec_time_ns
    return full


kernel.last_exec_time_ns = None
